# revision 47
# baseline (speedup 1.0000x reference)
"""ACT-R activation recurrence kernel for 8 TRN2 NeuronCores.

Math (per batch column b, S=128 steps):
  m[0] = -inf;  decay[j] = c*exp(m[j]) + a
  m[i] = log(sum_{j<i} ((sp[i]-sp[j])*scale)^(-decay[j])),  scale = 86400*h
  out[i-1] = sigmoid((m[i] - tau)/s)

Key reformulation: S_j := exp(m[j]) is the raw sum, so decay[j] = a + c*S_j —
no exp/log on the recurrence path.  Per term: d = sp_i - sp_j (PE, incremental
PSUM accumulation of broadcast gaps), l = Ln(scale*d) (ACT), q = l*(-decay_j)
(DVE, bf16), t = Exp(q) (ACT), masked sum over j<i (PE matmul with prefix-ones
stationary).  Final out = Sigmoid((Ln(S_i) - tau)/s) (ACT).

Batch axis (16384) is sharded 8 ways; each core runs an independent [128,2048]
recurrence; outputs are concatenated on host.  No collectives.
"""

import sys

for _p in ("/opt/trn_rl_repo",):
    if _p not in sys.path:
        sys.path.insert(0, _p)

import numpy as np
from contextlib import ExitStack

import concourse.bass as bass
import concourse.bacc as bacc
import concourse.tile as tile
from concourse import mybir
from concourse.bass_utils import run_bass_kernel_spmd

S = 128
B_FULL = 16384
N_CORES = 8
B = B_FULL // N_CORES  # 2048 per core

F32 = mybir.dt.float32
BF16 = mybir.dt.bfloat16
AF = mybir.ActivationFunctionType


def build_kernel(a, c, s, tau, h):
    scale = 86400.0 * float(h)
    nc = bacc.Bacc()

    # single packed input (one DMA = one queue proc → fewer drain waits):
    # cols [0:B]=sp, [B:2B]=gaps, [2B:2B+S]=negi
    inp_in = nc.declare_dram_parameter("inp", [S, 2 * B + S], F32, isOutput=False)
    out_ext = nc.declare_dram_parameter("out", [S - 1, B], F32, isOutput=True)

    NCH = B // 512  # psum-bank-sized column chunks

    with ExitStack() as ctx:
        tc = ctx.enter_context(tile.TileContext(nc))
        singles = ctx.enter_context(tc.tile_pool(name="singles", bufs=1))

        ALLIN = singles.tile([S, 2 * B + S], F32)
        nc.sync.dma_start(out=ALLIN[:], in_=inp_in[:])
        SP = ALLIN[:, 0:B]
        GAPS = ALLIN[:, B : 2 * B]
        NEGI = ALLIN[:, 2 * B : 2 * B + S]

        F16 = mybir.dt.float16
        # T persists and is zero-initialized once: at step i only rows [0,i)
        # are ever written, so rows [i,128) stay zero (valid prefix grows).
        # memset FIRST so later DVE waits cover it transitively.
        T = singles.tile([S, B], F16)
        nc.vector.memset(T[:], 0.0)
        ONESF = singles.tile([S, S], F32)
        nc.vector.memset(ONESF[:], 1.0)
        # ZC: zeros except column S (=128) = -c; slices ZC[:, S-i:2S-i] give a
        # one-hot-column stationary writing -c*sum into output row i only.
        ZC = singles.tile([S, 2 * S], F16)
        nc.vector.memset(ZC[:], 0.0)
        nc.vector.memset(ZC[:, S : S + 1], -float(c))
        AROW = singles.tile([1, B], F32)
        nc.vector.memset(AROW[:], -float(a))
        # persistent work tiles (bufs=1: pool slot rotation adds hidden deps
        # that overflow the 1-wait-per-instruction codegen limit)
        L = singles.tile([S, B], F32)
        Q = singles.tile([S, B], F16)
        # carrier scratch: ping-pong tile pairs so each carrier's WAW hazard is
        # at distance 2, already covered by the real ops' ridden own-waits
        # (Tile's dep tracking is per-tile, not per-range)
        JSCRa = singles.tile([1, S], F32)
        JSCRb = singles.tile([1, S], F32)
        CMSCRa = singles.tile([1, S], F32)
        CMSCRb = singles.tile([1, S], F32)
        ALSCRa = singles.tile([1, S], F32)
        ALSCRb = singles.tile([1, S], F32)
        AESCRa = singles.tile([1, S], F16)
        AESCRb = singles.tile([1, S], F16)
        J0SCRa = singles.tile([1, S], F16)
        J0SCRb = singles.tile([1, S], F16)
        A0SCRa = singles.tile([1, S], F32)
        A0SCRb = singles.tile([1, S], F32)
        JSCR = [JSCRa, JSCRb]
        CMSCR = [CMSCRa, CMSCRb]
        ALSCR = [ALSCRa, ALSCRb]
        AESCR = [AESCRa, AESCRb]
        J0SCR = [J0SCRa, J0SCRb]
        A0SCR = [A0SCRa, A0SCRb]
        BIASSIG = singles.tile([S, 1], F32)
        nc.vector.memset(BIASSIG[:], -float(tau) / float(s))

        psum_d = ctx.enter_context(nc.psum_tensor([S, B], F32))  # 4 banks
        # psum_S row j = -(a + c*S_j)  (negdecay, written directly by PE)
        psum_S = ctx.enter_context(nc.psum_tensor([S, B], F32))  # 4 banks

        # init psum_S = -a everywhere (opens the long accumulation group)
        for cc in range(NCH):
            sl = slice(512 * cc, 512 * (cc + 1))
            nc.tensor.matmul(
                psum_S[:, sl], ONESF[0:1, :], AROW[0:1, sl],
                start=True, stop=False, skip_group_check=True,
            )

        # PE matmuls can carry at most ONE semaphore wait; warm up the input
        # DMA dependency with a tiny throwaway matmul so later matmuls need at
        # most one new wait each (f32 forbids standalone ldweights).
        nc.tensor.matmul(psum_d[:, 0:1], ONESF[0:1, :], GAPS[0:1, 0:1],
                         start=True, stop=True, skip_group_check=True)

        for i in range(1, S):
            # d[j,b] = sum_{k<=i} gaps[k,b] - sp[j,b] = sp_i[b] - sp_j[b]
            for cc in range(NCH):
                sl = slice(512 * cc, 512 * (cc + 1))
                nc.tensor.matmul(
                    psum_d[:, sl], ONESF[0 : i + 1, :], GAPS[0 : i + 1, sl],
                    start=True, stop=False,
                )
                nc.tensor.matmul(
                    psum_d[:, sl], NEGI[:, :], SP[:, sl],
                    start=False, stop=True,
                )
            p = i % 2
            # ACT pre-carrier: absorb ACT-own tick (reads prev L) so the next
            # carrier holds only the PE tick.
            nc.scalar.copy(A0SCR[p][0:1, i : i + 1], L[0:1, 0:1])
            # ACT carrier: absorb the PE tick (reads last d-mm chunk's output).
            nc.scalar.copy(ALSCR[p][0:1, i : i + 1], psum_d[0:1, B - 1 : B])
            nc.scalar.activation(L[0:i, :], psum_d[0:i, :], AF.Ln, scale=scale)
            # DVE pre-carrier: absorb DVE-own tick (reads prev Q).
            nc.vector.tensor_copy(J0SCR[p][0:1, i : i + 1], Q[0:1, 0:1])
            # DVE carriers: absorb PE tick (psum_S last chunk) and ACT tick
            # (Ln's L output) so the mul carries only its DVE-own wait.
            nc.vector.tensor_copy(JSCR[p][0:1, i : i + 1], psum_S[0:1, B - 1 : B])
            nc.vector.tensor_copy(CMSCR[p][0:1, i : i + 1], L[0:1, 0:1])
            nc.vector.tensor_mul(Q[0:i, :], L[0:i, :], psum_S[0:i, :])
            # ACT carrier: absorb the DVE tick (reads Q) so Exp carries only
            # its ACT-own (T WAW) wait.
            nc.scalar.copy(AESCR[p][0:1, i : i + 1], Q[0:1, 0:1])
            nc.scalar.activation(T[0:i, :], Q[0:i, :], AF.Exp)
            # joiner: absorb the ACT(Exp) wait into a throwaway LS so the sum
            # matmuls below carry only the DVE wait (1-wait limit on PE).
            nc.tensor.ldweights(T[:, 0:S])
            # psum_S row i += -c * sum_{j<i} T[j, b]; all other rows += 0
            for cc in range(NCH):
                sl = slice(512 * cc, 512 * (cc + 1))
                nc.tensor.matmul(
                    psum_S[:, sl], ZC[:, S - i : 2 * S - i], T[:, sl],
                    start=False, stop=(i == S - 1), skip_group_check=True,
                )

        # recover S_i = (negdecay + a)/(-c) exactly, then sigmoid((ln S - tau)/s)
        nc.vector.tensor_copy(J0SCR[0][0:1, 0:1], Q[0:1, 0:1])  # DVE-own tick
        nc.vector.tensor_copy(JSCR[0][0:1, 0:1], psum_S[0:1, B - 1 : B])  # PE tick
        SS = singles.tile([S, B], F32)
        nc.vector.tensor_scalar(
            out=SS[:], in0=psum_S[:],
            scalar1=float(a), scalar2=-1.0 / float(c),
            op0=mybir.AluOpType.add, op1=mybir.AluOpType.mult,
        )
        M = singles.tile([S, B], F32)
        nc.scalar.activation(M[:], SS[:], AF.Ln)
        O = singles.tile([S, B], F32)
        nc.scalar.activation(
            O[:], M[:], AF.Sigmoid, scale=1.0 / float(s), bias=BIASSIG[:]
        )
        nc.sync.dma_start(out=out_ext[:], in_=O[1:S, :])

    nc.compile()
    return nc


def make_in_maps(sp: np.ndarray) -> list:
    negi = (-np.eye(S, dtype=np.float32)).copy()
    in_maps = []
    for ci in range(N_CORES):
        shard = np.ascontiguousarray(sp[:, ci * B : (ci + 1) * B])
        gaps = np.empty_like(shard)
        gaps[0] = shard[0]
        gaps[1:] = shard[1:] - shard[:-1]
        packed = np.concatenate([shard, gaps, negi], axis=1)
        in_maps.append({"inp": np.ascontiguousarray(packed)})
    return in_maps


def kernel(sp: np.ndarray, w: np.ndarray) -> np.ndarray:
    sp = np.ascontiguousarray(sp, dtype=np.float32)
    w = np.asarray(w, dtype=np.float32)
    a, c, s, tau, h = (float(x) for x in w)

    nc = build_kernel(a, c, s, tau, h)
    in_maps = make_in_maps(sp)

    res = run_bass_kernel_spmd(nc, in_maps, core_ids=list(range(N_CORES)))
    outs = [res.results[ci]["out"] for ci in range(N_CORES)]
    return np.concatenate(outs, axis=1).astype(np.float32)


if __name__ == "__main__":
    rng = np.random.default_rng(0)
    spt = np.cumsum(rng.uniform(0.1, 5.0, (S, B_FULL)).astype(np.float32), axis=0)
    wt = np.asarray(
        [0.176786766570677, 0.216967308403809, 0.254893976981164,
         -0.704205679427144, 0.025], dtype=np.float32)
    o = kernel(spt, wt)
    print(o.shape, o.dtype, o[:3, :3])
